# revision 20
# baseline (speedup 1.0000x reference)
"""Trainium2 Bass kernel for nn_Convolution_1176821039249.

Computes out = base_map * mean_k box_k(x) for k in {3,5,7,9,11,13,15} with
replicate padding, on 8 NeuronCores, row-sharded with a 7-row halo.

Algorithm (per core):
  The total 2D kernel K(di,dj) = sum_k 1/(7k^2) * 1[|di|<=k//2] 1[|dj|<=k//2]
  is decomposed over the horizontal "wing" basis
      T_0 = x(center),  T_m(j) = x(j-m) + x(j+m)   (m = 1..7)
  so that  out = sum_{b=0..7} P_b-vertical-band applied to T_b, where
      P_b(d) = sum_{k: k//2 >= max(b,|d|)} 1/(7k^2).
  Wings are one bf16 tensor_tensor add each on DVE (2x mode); the vertical
  pyramid bands are 8 PSUM-accumulated banded matmuls on the PE per tile;
  ACT drains PSUM, GPSIMD multiplies by base_map.
"""

import os
import numpy as np
import ml_dtypes

F16 = np.float16

H = W = 4096
PAD = 7
N_CORES = 8
RPC = H // N_CORES          # 512 output rows per core
TILE_M = 114                # output rows per row tile (128 - 2*PAD)
N_TILES = 5                 # 4 * 114 + 56 = 512
LAST_M = RPC - 4 * TILE_M   # 56
STRIP = 2048                # output cols per strip
N_STRIPS = W // STRIP       # 2
CHUNK = 512                 # matmul N chunk (one PSUM bank of fp32)
KERNEL_SIZES = (3, 5, 7, 9, 11, 13, 15)

_CACHE = {}


def _bands_np() -> np.ndarray:
    """lhsT band matrices, [128, 8*TILE_M] bf16.

    Band b column i row p holds P_b(p - i - 7): the vertical pyramid profile
    applied to wing tensor T_b.
    """
    w = {k: 1.0 / (7.0 * k * k) for k in KERNEL_SIZES}
    P = np.zeros((8, 15), dtype=np.float64)
    for b in range(8):
        for d in range(-7, 8):
            P[b, d + 7] = sum(w[k] for k in KERNEL_SIZES if k // 2 >= max(b, abs(d)))
    M = np.zeros((128, 8 * TILE_M), dtype=np.float64)
    for b in range(8):
        for i in range(TILE_M):
            p_lo = i  # d = p - i - 7 in [-7, 7]; P is indexed at d + 7 = p - i
            for p in range(p_lo, p_lo + 15):
                M[p, b * TILE_M + i] = P[b, p - i]
    return M.astype(F16)


def _build_nc():
    import concourse.bass as bass
    import concourse.mybir as mybir
    import concourse.tile as tile

    dt = mybir.dt
    SHARD_R = RPC + 2 * PAD     # 526
    SHARD_C = W + 2 * PAD       # 4110
    SW = STRIP + 2 * PAD        # 2062 input cols per strip

    nc = bass.Bass()
    xb_d = nc.declare_dram_parameter("xb", [SHARD_R, SHARD_C], dt.float16, isOutput=False)
    base_d = nc.declare_dram_parameter("base", [RPC, W], dt.float16, isOutput=False)
    bands_d = nc.declare_dram_parameter("bands", [128, 8 * TILE_M], dt.float16, isOutput=False)
    out_d = nc.declare_dram_parameter("out", [RPC, W], dt.float32, isOutput=True)

    with tile.TileContext(nc) as tc:
        with (
            tc.tile_pool(name="const", bufs=1) as constp,
            tc.tile_pool(name="xin", bufs=2) as xpool,
            tc.tile_pool(name="wings", bufs=2) as apool,
            tc.tile_pool(name="io", bufs=2) as iopool,
            tc.tile_pool(name="psum", bufs=2, space="PSUM") as psump,
        ):
            bands_sb = constp.tile([128, 8 * TILE_M], dt.float16, name="bands_sb")
            nc.sync.dma_start(bands_sb[:], bands_d[:])

            for t in range(N_TILES):
                M = TILE_M if t < N_TILES - 1 else LAST_M
                K = M + 2 * PAD
                r0 = t * TILE_M
                # Full-width input tile; ACT makes two copies so that (a) the
                # xt-load DMA has a single consuming engine (DIRECT2D DMAs
                # support very few sync waits) and (b) wing reads stay
                # 4B-aligned in every parity (fp16 2x_1p DVE mode needs
                # aligned packed operands): xc is aligned, x2 shifted by one.
                # one dedicated slot per row tile: xt loads never reuse a slot, so
                # the HWDGE load DMAs carry no sync waits (1-wait DMA limit)
                xt = xpool.tile([128, SHARD_C], dt.float16, tag="xt", name="xt", bufs=N_TILES)
                nc.sync.dma_start(xt[:K, :], xb_d[r0:r0 + K, :])
                xc = xpool.tile([128, SHARD_C], dt.float16, tag="xc", name="xc")
                x2 = xpool.tile([128, SHARD_C - 1], dt.float16, tag="x2", name="x2")
                # tiny guard writes: absorb the slots' WAR waits (DVE/PE
                # readers of the previous tenants) on cheap ACT ops, so the
                # real copies carry at most one sync wait each (walrus caps
                # sync waits per instruction)
                nc.scalar.copy(x2[0:1, 0:2], bands_sb[0:1, 0:2])
                nc.scalar.copy(xc[0:1, 0:2], bands_sb[0:1, 0:2])
                nc.scalar.copy(xc[:K, :], xt[:K, :])
                nc.scalar.copy(x2[:K, :], xt[:K, 1:SHARD_C])
                # fresh slot per tile (like xt): the load carries no sync waits
                bt = iopool.tile([128, W], dt.float16, tag="bt", name="bt", bufs=N_TILES)
                nc.sync.dma_start(bt[:M, :], base_d[r0:r0 + M, :])

                for s in range(N_STRIPS):
                    c0 = s * STRIP
                    wings = []
                    tiles = [apool.tile([128, STRIP], dt.float16, tag=f"a{m}", name=f"a{m}")
                             for m in range(1, 8)]
                    # guard: absorb the max PE slot-release tick (band 7 reads
                    # its wing last) once on DVE, so the wing adds below don't
                    # each carry a PE sync wait
                    nc.vector.tensor_copy(tiles[6][0:1, 0:2], bands_sb[0:1, 0:2])
                    for m in range(1, 8):
                        a = tiles[m - 1]
                        if m % 2 == 1:
                            nc.vector.tensor_add(
                                a[:K, :],
                                xc[:K, c0 + PAD - m:c0 + PAD - m + STRIP],
                                xc[:K, c0 + PAD + m:c0 + PAD + m + STRIP],
                            )
                        else:
                            # x2[:, c] == xt[:, c+1]; offsets stay even
                            nc.vector.tensor_add(
                                a[:K, :],
                                x2[:K, c0 + PAD - 1 - m:c0 + PAD - 1 - m + STRIP],
                                x2[:K, c0 + PAD - 1 + m:c0 + PAD - 1 + m + STRIP],
                            )
                        wings.append(a)

                    ps = psump.tile([128, STRIP], dt.float32, tag="ps", name="ps")
                    for b in range(8):
                        # center term reads x2 (== xt shifted by 1) so the PE
                        # is not a direct consumer of xt
                        rhs = (x2[:K, c0 + PAD - 1:c0 + PAD - 1 + STRIP]
                               if b == 0 else wings[b - 1][:K, :])
                        lhsT = bands_sb[:K, b * TILE_M:b * TILE_M + M]
                        for c in range(STRIP // CHUNK):
                            nc.tensor.matmul(
                                ps[:M, c * CHUNK:(c + 1) * CHUNK],
                                lhsT,
                                rhs[:, c * CHUNK:(c + 1) * CHUNK],
                                start=(b == 0),
                                stop=(b == 7),
                            )

                    # Pool compute instructions only support a single sync
                    # wait, so both multiply operands must come from one
                    # engine: ACT drains PSUM and also stages base_map.
                    btc = iopool.tile([128, STRIP], dt.float32, tag="btc", name="btc")
                    nc.scalar.copy(btc[:M, :], bt[:M, c0:c0 + STRIP])
                    acc = iopool.tile([128, STRIP], dt.float32, tag="acc", name="acc")
                    nc.scalar.copy(acc[:M, :], ps[:M, :])
                    nc.gpsimd.tensor_mul(acc[:M, :], acc[:M, :], btc[:M, :])
                    nc.gpsimd.dma_start(out_d[r0:r0 + M, c0:c0 + STRIP], acc[:M, :])
    return nc


def _split_sync_waits(nc):
    """Walrus codegen only supports one sync wait per instruction; hoist
    extra waits of engine instructions onto injected same-engine NoOps
    (identical semantics: the sequencer blocks at the NoOp first).  DMA
    instructions are left alone — they ride DGE queues, where walrus
    drops same-queue waits and our DMAs carry at most one besides those.
    """
    import concourse.mybir as mybir

    n_nops = 0
    for fn in nc.m.functions:
        for bb in fn.blocks:
            new = []
            for inst in bb.instructions:
                si = inst.sync_info
                if (si is not None and si.on_wait and len(si.on_wait) > 1
                        and not isinstance(inst, mybir.InstDMACopy)):
                    waits = list(si.on_wait)
                    for w in waits[:-1]:
                        nop = mybir.InstNoOp(name=f"{inst.name}-w{n_nops}", ins=[], outs=[])
                        nop.engine = inst.engine
                        nop.sync_info = mybir.SyncInfo(on_wait=[w], on_update=[])
                        new.append(nop)
                        n_nops += 1
                    inst.sync_info = mybir.SyncInfo(
                        on_wait=[waits[-1]], on_update=list(si.on_update))
                new.append(inst)
            bb.instructions = new
    return n_nops


def _get_nc():
    if "nc" not in _CACHE:
        nc = _build_nc()
        _split_sync_waits(nc)
        _CACHE["nc"] = nc
    return _CACHE["nc"]


def _run(x: np.ndarray, base_map: np.ndarray, trace: bool = False):
    from concourse.bass_utils import run_bass_kernel_spmd

    nc = _get_nc()
    xp = np.pad(np.asarray(x, dtype=np.float32), PAD, mode="edge").astype(F16)
    base_map = np.ascontiguousarray(np.asarray(base_map, dtype=np.float32).astype(F16))
    bands = _bands_np()
    in_maps = []
    for c in range(N_CORES):
        r0 = c * RPC
        in_maps.append({
            "xb": np.ascontiguousarray(xp[r0:r0 + RPC + 2 * PAD]),
            "base": base_map[r0:r0 + RPC],
            "bands": bands,
        })
    res = run_bass_kernel_spmd(nc, in_maps, list(range(N_CORES)), trace=trace)
    out = np.concatenate([res.results[c]["out"] for c in range(N_CORES)], axis=0)
    return out[None, None].astype(np.float32), res


def kernel(x: np.ndarray, base_map: np.ndarray) -> np.ndarray:
    out, _ = _run(x, base_map, trace=False)
    return out
